# revision 17
# baseline (speedup 1.0000x reference)
"""BiasedMHA + GLU fused Trainium2 kernel.

Problem: out = GLU(x) + OutProj(MHA(x, attn_bias))  with
  B=8, N=1024, D=768, H=12, HD=64, fp32 inputs/outputs.

Strategy: data-parallel over batch across the 8 NeuronCores (one batch
element per core, no collectives). Per core everything is computed in a
"transposed" [channel, token] layout so every GEMM contracts the
partition dimension without any on-device activation transposes:

  xT [D, N] (host-pretransposed)    qT/kT = W.T-stationary GEMMs  [D, N]
  v via xT-stationary GEMM          -> natural [token, head*65] layout
  scoresT[k, q] = kT_h.T @ qT_h accumulated on top of PE-transposed
  attn_bias blocks (bf16 identity-matmuls straight into the scores PSUM)
  softmax over k (= partitions): no max-subtraction (|scores| <= ~8),
  denominator via an appended ones-column in v, applied after PV.
  Matmuls run in float32r (TF32-like, 11-bit mantissa, 4x faster than
  fp32 on the PE); measured end-to-end error vs fp32 reference ~6e-4.
"""

import os
import sys

for _p in ("/opt/trn_rl_repo", "/root/.axon_site/_ro/trn_rl_repo"):
    if os.path.isdir(_p) and _p not in sys.path:
        sys.path.insert(0, _p)

import numpy as np
import ml_dtypes

import concourse.bacc as bacc
import concourse.mybir as mybir
from concourse import tile
from concourse.bass_utils import run_bass_kernel_spmd
from concourse.masks import make_identity

B, N, D, H, HD = 8, 1024, 768, 12, 64
P = 128
ND = D // P           # 6 channel tiles
NN = N // P           # 8 token tiles
VW = H * (HD + 1)     # 780: v layout [token, h*(64+1)] with ones column

F32 = mybir.dt.float32
F32R = mybir.dt.float32r
BF16 = mybir.dt.bfloat16
AF = mybir.ActivationFunctionType
OP = mybir.AluOpType


def _r32r(x):
    """Round fp32 array to float32r (1s + 8e + 11m, RNE)."""
    u = np.ascontiguousarray(x, dtype=np.float32).view(np.uint32)
    odd = (u >> np.uint32(12)) & np.uint32(1)
    u = (u + np.uint32(0x7FF) + odd) & np.uint32(0xFFFFF000)
    return u.view(np.float32)


def _emit(nc, tc, xT, biasb, w, bvec, outT, dbg=None):
    with tc.tile_pool(name="const", bufs=1) as constp, \
         tc.tile_pool(name="qkvT", bufs=1) as qkvp, \
         tc.tile_pool(name="ctxT", bufs=1) as ctxp_sb:

        ident = constp.tile([P, P], BF16, tag="ident", name="ident")
        make_identity(nc, ident[:])
        if dbg is not None:
            idf = constp.tile([P, P], F32, tag="idf", name="idf", bufs=1)
            nc.vector.tensor_copy(idf[:], ident[:])
            nc.sync.dma_start(dbg["ident"], idf[:])
        bvt = {}
        for nm in ("bq", "bk", "bo"):
            t = constp.tile([P, ND], F32, tag=f"t{nm}", name=f"t{nm}")
            nc.sync.dma_start(t[:], bvec[nm].ap().rearrange("(j p) -> p j", p=P))
            bvt[nm] = t
        ones12 = constp.tile([P, H], F32, tag="ones12", name="ones12")
        nc.vector.memset(ones12[:], 1.0)

        qT = [qkvp.tile([P, N], F32R, tag=f"qT{i}", name=f"qT{i}") for i in range(ND)]
        kT = [qkvp.tile([P, N], F32R, tag=f"kT{i}", name=f"kT{i}") for i in range(ND)]
        vsb = [qkvp.tile([P, VW], F32R, tag=f"v{t}", name=f"v{t}") for t in range(NN)]
        vv = [t.rearrange("p (h c) -> p h c", c=HD + 1) for t in vsb]
        ctxT = [ctxp_sb.tile([P, N], F32R, tag=f"cT{i}", name=f"cT{i}") for i in range(ND)]

        # Early bias prefetch: qq=0, K=0 staging tiles, DMA'd from t=0 so the
        # PE never stalls at the B->C phase transition (HAM stays warm).
        KH = N * H // 2  # 6144 bf16 = half of k, all heads
        with tc.tile_pool(name="stage0", bufs=2) as stp0:
            stg00 = []
            for j in range(2):
                st = stp0.tile([P, KH], BF16, tag="stage0", name="stage0")
                nc.sync.dma_start(st[:], biasb[j * P:(j + 1) * P, 0:KH])
                stg00.append(st.rearrange("p (k h) -> p k h", h=H))

            # ---------------- Phase B: q/k/v projections ----------------
            with tc.tile_pool(name="xTw", bufs=1) as xwp, \
                 tc.tile_pool(name="psB", bufs=4, space="PSUM") as psB:
                xsb = [xwp.tile([P, N], F32R, tag=f"x{i}", name=f"x{i}") for i in range(ND)]
                for i in range(ND):
                    nc.sync.dma_start(xsb[i][:], xT[i * P:(i + 1) * P, :])
                wsb = {}
                for nm in ("wq", "wk", "wv"):
                    wsb[nm] = [xwp.tile([P, D], F32R, tag="wpool", name=f"{nm}{i}",
                                        bufs=12) for i in range(ND)]
                    for i in range(ND):
                        nc.sync.dma_start(wsb[nm][i][:], w[nm][i * P:(i + 1) * P, :])

                # qT/kT: out[j, c] = sum_i w*T[i][:, j].T @ xT[i][:, c]  (+bias)
                for nm, dst in (("wq", qT), ("wk", kT)):
                    for j in range(ND):
                        for c in range(2):
                            ps = psB.tile([P, 512], F32, tag="psB", name="psB")
                            for i in range(ND):
                                nc.tensor.matmul(
                                    ps[:], wsb[nm][i][:, j * P:(j + 1) * P],
                                    xsb[i][:, c * 512:(c + 1) * 512],
                                    start=(i == 0), stop=(i == ND - 1))
                            bt = bvt["bq" if nm == "wq" else "bk"]
                            nc.scalar.activation(dst[j][:, c * 512:(c + 1) * 512],
                                                 ps[:], AF.Identity,
                                                 bias=bt[:, j:j + 1])

                # v: xT-stationary GEMM -> [token, dout], strided into vsb
                for t in range(NN):
                    nc.vector.tensor_copy(vv[t][:, :, HD], ones12[:])
                    for c, (lo, sz) in enumerate(((0, 512), (512, 256))):
                        ps = psB.tile([P, 512], F32, tag="psB", name="psB")
                        for i in range(ND):
                            nc.tensor.matmul(
                                ps[:, 0:sz], xsb[i][:, t * P:(t + 1) * P],
                                wsb["wv"][i][:, lo:lo + sz],
                                start=(i == 0), stop=(i == ND - 1))
                        h0 = lo // HD
                        nc.vector.tensor_copy(
                            vv[t][:, h0:h0 + sz // HD, 0:HD],
                            ps[:, 0:sz].rearrange("p (h c) -> p h c", c=HD))

            if dbg is not None:
                nc.sync.dma_start(dbg["qT0"], qT[0][:].bitcast(F32))
                nc.sync.dma_start(dbg["kT0"], kT[0][:].bitcast(F32))
                nc.sync.dma_start(dbg["v0"], vsb[0][:].bitcast(F32))

            # ---------------- Phase C: attention ----------------
            with tc.tile_pool(name="stage", bufs=4) as stp, \
                 tc.tile_pool(name="expT", bufs=3) as expp, \
                 tc.tile_pool(name="norm", bufs=4) as normp, \
                 tc.tile_pool(name="psS", bufs=3, space="PSUM") as psS, \
                 tc.tile_pool(name="psC", bufs=2, space="PSUM") as psC:
                for qq in range(4):
                    stg = {}
                    for j in range(2):
                        qrow = (qq * 2 + j) * P
                        for K in range(2):
                            if qq == 0 and K == 0:
                                stg[(j, K)] = stg00[j]
                                continue
                            # K=0 tiles rotate through the early pool's 2
                            # slots; K=1 tiles through the main pool's 4.
                            pool = stp0 if K == 0 else stp
                            st = pool.tile([P, KH], BF16, name="stage",
                                           tag="stage0" if K == 0 else "stage")
                            nc.sync.dma_start(
                                st[:], biasb[qrow:qrow + P, K * KH:(K + 1) * KH])
                            stg[(j, K)] = st.rearrange("p (k h) -> p k h", h=H)
                    for h in range(12):
                        ht, hp = h // 2, (h % 2) * HD
                        s2 = [psS.tile([P, 1024], F32, tag="psS", name="psS")
                              for _ in range(2)]
                        for kt in range(NN):
                            bank, off = s2[kt // 4], (kt % 4) * 256
                            # qk first with start=True: a start=True clears
                            # has_written for the WHOLE bank, so it must precede
                            # the two bias-transpose sub-writes (start=False).
                            nc.tensor.matmul(
                                bank[:, off:off + 256],
                                kT[ht][hp:hp + HD, kt * P:(kt + 1) * P],
                                qT[ht][hp:hp + HD, qq * 256:(qq + 1) * 256],
                                start=True, stop=False)
                            for j in range(2):
                                kl = (kt % 4) * P
                                nc.tensor.matmul(
                                    bank[:, off + j * P: off + (j + 1) * P],
                                    stg[(j, kt // 4)][:, kl:kl + P, h],
                                    ident[:], start=False, stop=(j == 1))
                        ctx = psC.tile([HD + 1, 256], F32, tag="psC", name="psC")
                        exps = []
                        for bi in range(2):
                            e = expp.tile([P, 1024], F32R, tag="expT", name="expT")
                            nc.scalar.activation(e[:], s2[bi][:], AF.Exp)
                            exps.append(e)
                        if dbg is not None and qq == 0 and h == 0:
                            sc0 = expp.tile([P, 512], F32, tag="sc0", name="sc0", bufs=1)
                            nc.vector.tensor_copy(sc0[:], s2[0][:, 0:512])
                            nc.sync.dma_start(dbg["scores00"], sc0[:])
                            nc.sync.dma_start(dbg["exp00"], exps[0][:, 0:512].bitcast(F32))
                            st0 = expp.tile([P, 512], F32, tag="st0", name="st0", bufs=1)
                            nc.vector.tensor_copy(st0[:], stg[(0, 0)][:, 0:80, :].rearrange("p k h -> p (k h)")[:, 0:512])
                            nc.sync.dma_start(dbg["stage00"], st0[:])
                        for kt in range(NN):
                            nc.tensor.matmul(
                                ctx[:], vsb[kt][:, h * (HD + 1):(h + 1) * (HD + 1)],
                                exps[kt // 4][:, (kt % 4) * 256:(kt % 4) * 256 + 256],
                                start=(kt == 0), stop=(kt == NN - 1))
                        if dbg is not None and qq == 0 and h == 0:
                            cx0 = expp.tile([HD + 1, 256], F32, tag="cx0", name="cx0", bufs=1)
                            nc.vector.tensor_copy(cx0[:], ctx[:])
                            nc.sync.dma_start(dbg["ctx00"], cx0[:])
                        rowt = normp.tile([1, 256], F32, tag="rowt", name="rowt")
                        nc.scalar.activation(rowt[:], ctx[HD:HD + 1, :], AF.Copy)
                        rec = normp.tile([1, 256], F32, tag="rec", name="rec")
                        nc.vector.reciprocal_approx_fast(rec[:], rowt[:])
                        bc = normp.tile([HD, 256], F32, tag="bc", name="bc")
                        nc.gpsimd.partition_broadcast(bc[:], rec[:])
                        nc.vector.tensor_tensor(
                            ctxT[ht][hp:hp + HD, qq * 256:(qq + 1) * 256],
                            ctx[0:HD, :], bc[:], OP.mult)

        # ---------------- Phase D/E: out-proj + GLU gate + combine ----------------
        with tc.tile_pool(name="wDE", bufs=1) as wde, \
             tc.tile_pool(name="outb", bufs=3) as outb, \
             tc.tile_pool(name="psD", bufs=4, space="PSUM") as psD:
            x2 = [wde.tile([P, N], F32R, tag=f"x2{i}", name=f"x2{i}") for i in range(ND)]
            for i in range(ND):
                nc.sync.dma_start(x2[i][:], xT[i * P:(i + 1) * P, :])
            wo = [wde.tile([P, D], F32R, tag=f"wo{i}", name=f"wo{i}") for i in range(ND)]
            wg = [wde.tile([P, D], F32R, tag=f"wg{i}", name=f"wg{i}") for i in range(ND)]
            for i in range(ND):
                nc.sync.dma_start(wo[i][:], w["wo"][i * P:(i + 1) * P, :])
                nc.sync.dma_start(wg[i][:], w["wg"][i * P:(i + 1) * P, :])
            bgt = wde.tile([P, ND], F32, tag="bg", name="bg")
            nc.sync.dma_start(bgt[:], bvec["bg"].ap().rearrange("(j p) -> p j", p=P))
            xh = [wde.tile([P, N], F32, tag=f"xh{i}", name=f"xh{i}") for i in range(ND)]
            for i in range(ND):
                nc.vector.tensor_scalar_mul(xh[i][:], x2[i][:], 0.5)

            for j in range(ND):
                for c in range(2):
                    sl = slice(c * 512, (c + 1) * 512)
                    pg = psD.tile([P, 512], F32, tag="psD")
                    for i in range(ND):
                        nc.tensor.matmul(pg[:], wg[i][:, j * P:(j + 1) * P],
                                         x2[i][:, sl],
                                         start=(i == 0), stop=(i == ND - 1))
                    po = psD.tile([P, 512], F32, tag="psD")
                    for i in range(ND):
                        nc.tensor.matmul(po[:], wo[i][:, j * P:(j + 1) * P],
                                         ctxT[i][:, sl],
                                         start=(i == 0), stop=(i == ND - 1))
                    th = outb.tile([P, 512], F32, tag="tanh", name="tanh")
                    nc.scalar.activation(th[:], pg[:], AF.Tanh,
                                         bias=bgt[:, j:j + 1], scale=0.5)
                    u = outb.tile([P, 512], F32, tag="u", name="u")
                    nc.vector.scalar_tensor_tensor(
                        u[:], in0=th[:], scalar=1.0, in1=xh[j][:, sl],
                        op0=OP.add, op1=OP.mult)
                    fin = outb.tile([P, 512], F32, tag="fin", name="fin")
                    nc.vector.scalar_tensor_tensor(
                        fin[:], in0=po[:], scalar=bvt["bo"][:, j:j + 1], in1=u[:],
                        op0=OP.add, op1=OP.add)
                    nc.sync.dma_start(outT[j * P:(j + 1) * P, sl], fin[:])


_cache = {}


def _build(debug=False):
    key = ("nc", debug)
    if key in _cache:
        return _cache[key]
    nc = bacc.Bacc("TRN2", target_bir_lowering=False, debug=False, num_devices=8)
    xT = nc.dram_tensor("xT", [D, N], F32R, kind="ExternalInput")
    biasb = nc.dram_tensor("biasb", [N, N * H], BF16, kind="ExternalInput")
    w = {nm: nc.dram_tensor(nm, [D, D], F32R, kind="ExternalInput")
         for nm in ("wq", "wk", "wv", "wg", "wo")}
    bvec = {nm: nc.dram_tensor(nm, [D], F32, kind="ExternalInput")
            for nm in ("bq", "bk", "bg", "bo")}
    outT = nc.dram_tensor("outT", [D, N], F32, kind="ExternalOutput")
    dbg = None
    if debug:
        shapes = {"ident": [P, P], "scores00": [P, 512], "exp00": [P, 512],
                  "stage00": [P, 512], "ctx00": [HD + 1, 256],
                  "qT0": [P, N], "kT0": [P, N], "v0": [P, VW]}
        dbg = {nm: nc.dram_tensor("dbg_" + nm, sh, F32, kind="ExternalOutput").ap()
               for nm, sh in shapes.items()}
    with tile.TileContext(nc) as tc:
        _emit(nc, tc, xT.ap(), biasb.ap(), {k: v.ap() for k, v in w.items()},
              bvec, outT.ap(), dbg=dbg)
    nc.compile()
    _cache[key] = nc
    return nc


def _prep(inputs):
    scaling = HD ** (-0.5)
    shared = {
        "wq": _r32r(inputs["Wq"].T * scaling),
        "wk": _r32r(inputs["Wk"].T),
        "wv": _r32r(inputs["Wv"].T),
        "wg": _r32r(inputs["Wg"].T),
        "wo": _r32r(inputs["Wo"].T),
        "bq": np.ascontiguousarray(inputs["bq"] * scaling, np.float32),
        "bk": np.ascontiguousarray(inputs["bk"], np.float32),
        "bg": np.ascontiguousarray(inputs["bg"], np.float32),
        "bo": np.ascontiguousarray(
            inputs["bo"] + inputs["Wo"] @ inputs["bv"], np.float32),
    }
    ab = np.ascontiguousarray(inputs["attn_bias"], np.float32)
    nd = np.ascontiguousarray(inputs["ndata"], np.float32)
    in_maps = []
    for b in range(B):
        m = dict(shared)
        m["xT"] = _r32r(nd[b].T)
        m["biasb"] = ab[b].reshape(N, N * H).astype(ml_dtypes.bfloat16)
        in_maps.append(m)
    return in_maps


def run(inputs, trace=False, debug=False, **kw):
    nc = _build(debug=debug)
    in_maps = _prep(inputs)
    res = run_bass_kernel_spmd(nc, in_maps, core_ids=list(range(B)),
                               trace=trace, **kw)
    out = np.stack([np.ascontiguousarray(r["outT"].T) for r in res.results])
    return out, res


def kernel(**inputs):
    out, _ = run(inputs)
    return out
